# revision 10
# baseline (speedup 1.0000x reference)
"""Trainium2 Bass kernel for the VQ-VAE-ish reference net (vq_codebook).

Data-parallel: 8 images -> 8 NeuronCores, one image per core. Inside each
core the full encoder -> VQ -> decoder chain runs as tap-matmul convolutions
(channels on partitions, weights stationary, shifted-image APs as the moving
operand), with GroupNorm stats fused into the producing conv's epilogue.

PE-array packing: the input tile is replicated into partitions 64..127
shifted by one column (SBUF->SBUF DMA), so two horizontal taps contract in a
single K=128 matmul; two output row-tiles run concurrently in the left/right
array halves via tile_position=(0,0)/(0,64). k=4 (in_conv) packs perfectly
(8 pair-matmuls); k=3 packs 9 taps into 6 matmul slots.

Precision: encoder + VQ in fp32 (argmin flips are catastrophic: 1 flip costs
~1e-2 output rel err), decoder in bf16 (measured ~7e-3 rel err).
"""
import sys

for _p in ("/opt/trn_rl_repo", "/root/.axon_site/_ro/trn_rl_repo"):
    if _p not in sys.path:
        sys.path.insert(0, _p)

import numpy as np
import concourse.bass as bass
import concourse.mybir as mybir
from concourse.bass_utils import run_bass_kernel_spmd
from concourse.tile import TileContext
from concourse.vector_clock import ScopedClock
from concourse.bass_primitives_rust import SemaphoreHandle
from concourse.masks import make_identity

F32 = mybir.dt.float32
BF16 = mybir.dt.bfloat16
AX = mybir.AxisListType
ALU = mybir.AluOpType
ACT = mybir.ActivationFunctionType

C = 64
C2 = 128
GROUPS = 32
EPS = 1e-6
N_CORES = 8

# ---------------------------------------------------------------------------
# walrus in this container rejects >1 sync wait per instruction: split excess
# waits into standalone InstEventSemaphore instructions after Tile scheduling.
# ---------------------------------------------------------------------------
WAIT_LIMIT = 1
_split_counter = [0]


def _split_excess_waits(nc, limit=WAIT_LIMIT):
    f = nc.m.functions[0]
    for bb in f.blocks:
        insts = bb.instructions
        out = []
        changed = False
        for inst in insts:
            si = inst.sync_info
            if si is not None and len(si.on_wait) > limit:
                waits = list(si.on_wait)
                extra, keep = waits[: len(waits) - limit], waits[len(waits) - limit:]
                for w in extra:
                    _split_counter[0] += 1
                    nop = mybir.InstEventSemaphore(
                        name=f"I-waitsplit-{_split_counter[0]}", ins=[], outs=[]
                    )
                    nop.engine = inst.engine
                    nop.sync_info = mybir.SyncInfo(on_wait=[w], on_update=[])
                    nc.register_instruction(nop, overwrite=True)
                    out.append(nop)
                inst.sync_info = mybir.SyncInfo(
                    on_wait=keep, on_update=list(si.on_update)
                )
                changed = True
            out.append(inst)
        if changed:
            bb.instructions = out


def _patched_drain_and_barrier(self, tick_clock, wait_clock):
    nc = self.nc
    drain_inst = nc.sync.drain()
    wait_clock.add_sem_waits(
        drain_inst.ins, ScopedClock({None: tick_clock.global_clock})
    )
    si = drain_inst.ins.sync_info
    waits = list(si.on_wait) if si is not None else []
    if len(waits) > 1:
        drain_inst.ins.sync_info = mybir.SyncInfo(
            on_wait=[], on_update=list(si.on_update)
        )
        for w in waits:
            nc.sync.wait_ge(SemaphoreHandle(w.ant_name, w.id), w.wait_value)
        nc.sync.drain()
    nc.all_engine_barrier()
    popped = nc._tile_sem_poison_stack.pop()
    assert popped is self._sem_poison
    nc.clear_and_free_semaphores(list(self.sems.allocated().values()))
    nc.all_engine_barrier()


def _install_patches():
    if getattr(TileContext, "_vqnet_patched", False):
        return
    TileContext._drain_and_barrier = _patched_drain_and_barrier
    _orig_exit = TileContext.__exit__

    def _patched_exit(self, exc_type, exc_val, exc_tb):
        r = _orig_exit(self, exc_type, exc_val, exc_tb)
        if exc_type is None:
            _split_excess_waits(self.nc)
        return r

    TileContext.__exit__ = _patched_exit
    TileContext._vqnet_patched = True


def _taps_layout(k):
    """pairs: [(ky, kx)] covering (ky,kx)+(ky,kx+1); singles: [(ky, k-1)]."""
    pairs = [(ky, kx) for ky in range(k) for kx in range(0, k - 1, 2)]
    singles = [(ky, k - 1) for ky in range(k)] if k % 2 == 1 else []
    return pairs, singles


# ---------------------------------------------------------------------------
# Builder
# ---------------------------------------------------------------------------

class Net:
    def __init__(self, nc, tc, pools):
        self.nc = nc
        self.tc = tc
        self.wpool, self.iopool, self.smallpool, self.psum, self.psum_small = pools

    # ---- weights -----------------------------------------------------------
    def conv_weights(self, name, k, dtype):
        """Packed weights: wp [C2, npair, C], ws [C, ns, C] (ns may be 0),
        bias duplicated to [C2, 1]."""
        nc = self.nc
        pairs, singles = _taps_layout(k)
        wp_ext = nc.declare_dram_parameter(
            f"{name}_wp", [C2, len(pairs), C], F32, isOutput=False
        )
        wp_s = self.wpool.tile([C2, len(pairs), C], dtype,
                               name=f"{name}_wp", tag=f"{name}_wp")
        if dtype == F32:
            nc.sync.dma_start(out=wp_s[:], in_=wp_ext.ap())
        else:
            wp_f = self.iopool.tile([C2, len(pairs), C], F32,
                                    name=f"{name}_wpf", tag="wstage", bufs=2)
            nc.sync.dma_start(out=wp_f[:], in_=wp_ext.ap())
            nc.vector.tensor_copy(out=wp_s[:], in_=wp_f[:])
        ws_s = None
        if singles:
            ws_ext = nc.declare_dram_parameter(
                f"{name}_ws", [C, len(singles), C], F32, isOutput=False
            )
            ws_s = self.wpool.tile([C, len(singles), C], dtype,
                                   name=f"{name}_ws", tag=f"{name}_ws")
            if dtype == F32:
                nc.sync.dma_start(out=ws_s[:], in_=ws_ext.ap())
            else:
                ws_f = self.iopool.tile([C, len(singles), C], F32,
                                        name=f"{name}_wsf", tag="wstage", bufs=2)
                nc.sync.dma_start(out=ws_f[:], in_=ws_ext.ap())
                nc.vector.tensor_copy(out=ws_s[:], in_=ws_f[:])
        b_ext = nc.declare_dram_parameter(f"{name}_b", [C], F32, isOutput=False)
        b_s = self.wpool.tile([C2, 1], F32, name=f"{name}_b", tag=f"{name}_b")
        nc.sync.dma_start(out=b_s[0:C, :], in_=b_ext.ap().unsqueeze(-1))
        nc.sync.dma_start(out=b_s[C:C2, :], in_=b_ext.ap().unsqueeze(-1))
        return (wp_s, ws_s, b_s, k)

    def gn_weights(self, name):
        nc = self.nc
        g_ext = nc.declare_dram_parameter(f"{name}_g", [C], F32, isOutput=False)
        b_ext = nc.declare_dram_parameter(f"{name}_bt", [C], F32, isOutput=False)
        g_s = self.wpool.tile([C2, 1], F32, name=f"{name}_g", tag=f"{name}_g")
        b_s = self.wpool.tile([C2, 1], F32, name=f"{name}_bt", tag=f"{name}_bt")
        nc.sync.dma_start(out=g_s[0:C, :], in_=g_ext.ap().unsqueeze(-1))
        nc.sync.dma_start(out=g_s[C:C2, :], in_=g_ext.ap().unsqueeze(-1))
        nc.sync.dma_start(out=b_s[0:C, :], in_=b_ext.ap().unsqueeze(-1))
        nc.sync.dma_start(out=b_s[C:C2, :], in_=b_ext.ap().unsqueeze(-1))
        return g_s, b_s

    # ---- GroupNorm scale/bias derivation ----------------------------------
    def gn_apply_params(self, name, stats, nstat, gamma, beta, pairmat2):
        """stats [C2, nstat, 6] -> s,t [C2,1] with y = s*x + t."""
        nc = self.nc
        sp = self.smallpool
        mv = sp.tile([C2, 2], F32, name=f"{name}_mv", tag=f"{name}_mv")
        nc.vector.bn_aggr(out=mv[:], in_=stats[:, 0:nstat, :])
        pk = sp.tile([C2, 2], F32, name=f"{name}_pk", tag=f"{name}_pk")
        nc.vector.tensor_copy(out=pk[:, 0:1], in_=mv[:, 0:1])
        nc.vector.tensor_tensor(
            out=pk[:, 1:2], in0=mv[:, 0:1], in1=mv[:, 0:1], op=ALU.mult
        )
        nc.vector.tensor_tensor(
            out=pk[:, 1:2], in0=pk[:, 1:2], in1=mv[:, 1:2], op=ALU.add
        )
        psm = self.psum_small.tile([C2, 2], F32, name="gn_psum", tag="gn_psum",
                                   bufs=1)
        nc.tensor.matmul(psm[:], pairmat2[:], pk[:], start=True, stop=True)
        gmean = sp.tile([C2, 1], F32, name=f"{name}_gmean", tag=f"{name}_gmean")
        nc.vector.tensor_copy(out=gmean[:], in_=psm[:, 0:1])
        varg = sp.tile([C2, 1], F32, name=f"{name}_varg", tag=f"{name}_varg")
        nc.vector.tensor_tensor(out=varg[:], in0=gmean[:], in1=gmean[:],
                                op=ALU.mult)
        nc.vector.tensor_tensor(out=varg[:], in0=psm[:, 1:2], in1=varg[:],
                                op=ALU.subtract)
        nc.scalar.activation(out=varg[:], in_=varg[:], func=ACT.Sqrt,
                             bias=self.eps_t[:], scale=1.0)
        nc.vector.reciprocal(out=varg[:], in_=varg[:])
        s_t = sp.tile([C2, 1], F32, name=f"{name}_s", tag=f"{name}_s")
        t_t = sp.tile([C2, 1], F32, name=f"{name}_t", tag=f"{name}_t")
        nc.vector.tensor_tensor(out=s_t[:], in0=gamma[:], in1=varg[:], op=ALU.mult)
        nc.vector.tensor_tensor(out=t_t[:], in0=gmean[:], in1=s_t[:], op=ALU.mult)
        nc.vector.tensor_tensor(out=t_t[:], in0=beta[:], in1=t_t[:],
                                op=ALU.subtract)
        return s_t, t_t

    # ---- packed conv pass --------------------------------------------------
    def conv_pass(
        self, name, src, dst, wpack,
        H_in, W_in, H_out, W_out, stride, dtype,
        gn=None, lrelu=False, residual=None, stats=None,
        rows_per_block=16, out_dtype=None,
    ):
        nc = self.nc
        wp_s, ws_s, b_s, k = wpack
        pairs, singles = _taps_layout(k)
        pad = 1
        Wp = W_in + 2 * pad
        out_dtype = out_dtype or dtype
        rpt = min(max(1, 512 // W_out), H_out)   # rows per psum tile
        assert rpt * W_out <= 512
        rows_per_block = max(rpt, (rows_per_block // rpt) * rpt)
        stat_i = 0
        n_slots = len(pairs) + len(singles)

        def emit_tile(tin, py0, ty, th, col):
            """Matmuls for one output row-tile into psum column half `col`."""
            base = col * C
            slot = 0
            for (ky, kx) in pairs + singles:
                is_pair = slot < len(pairs)
                r0 = stride * ty - py0 + ky
                npart = C2 if is_pair else C
                rhs = bass.AP(
                    tensor=tin.tensor,
                    offset=tin.offset + r0 * Wp + kx,
                    ap=[[tin.ap[0][0], npart], [stride * Wp, th],
                        [stride, W_out]],
                )
                lhsT = wp_s[:, slot, :] if is_pair else ws_s[:, slot - len(pairs), :]
                nc.tensor.matmul(
                    ps[base: base + C, 0:th, :], lhsT, rhs,
                    start=(slot == 0), stop=(slot == n_slots - 1),
                    tile_position=(0, base),
                    skip_group_check=True,
                )
                slot += 1

        def epilogue(tiles):
            """tiles: list of (ty, th, col). Emits bias/residual/stats/DMA."""
            nonlocal stat_i
            np_ = len(tiles) * C
            to = self.iopool.tile([C2, rpt, W_out], out_dtype,
                                  name=f"{name}_out", tag="conv_out", bufs=3)
            lo = tiles[0][2] * C
            hi = lo + np_
            assert lo == 0
            full = to[0:np_, 0: tiles[0][1], :]
            nc.scalar.activation(
                out=full, in_=ps[0:np_, 0: tiles[0][1], :],
                func=ACT.Identity, bias=b_s[0:np_, :], scale=1.0,
            ) if all(t[1] == tiles[0][1] for t in tiles) else None
            if not all(t[1] == tiles[0][1] for t in tiles):
                for (ty, th, col) in tiles:
                    b0 = col * C
                    nc.scalar.activation(
                        out=to[b0: b0 + C, 0:th, :], in_=ps[b0: b0 + C, 0:th, :],
                        func=ACT.Identity, bias=b_s[b0: b0 + C, :], scale=1.0,
                    )
            if residual is not None:
                tr = self.iopool.tile([C2, rpt, W_out], dtype,
                                      name=f"{name}_res", tag="conv_res", bufs=3)
                for (ty, th, col) in tiles:
                    b0 = col * C
                    nc.sync.dma_start(out=tr[b0: b0 + C, 0:th, :],
                                      in_=residual[:, ty: ty + th, :])
                    nc.vector.tensor_tensor(
                        out=to[b0: b0 + C, 0:th, :], in0=to[b0: b0 + C, 0:th, :],
                        in1=tr[b0: b0 + C, 0:th, :], op=ALU.add,
                    )
            if stats is not None:
                st_tile, _ = stats
                if len(tiles) == 2:
                    nc.vector.bn_stats(
                        out=st_tile[:, stat_i, :],
                        in_=to[:, 0: tiles[0][1], :].rearrange(
                            "p a b -> p (a b)"),
                    )
                else:
                    (ty, th, col) = tiles[0]
                    nc.vector.bn_stats(
                        out=st_tile[0:C, stat_i, :],
                        in_=to[0:C, 0:th, :].rearrange("p a b -> p (a b)"),
                    )
                    # duplicate lower-half stats into upper half (tiny DMA)
                    nc.sync.dma_start(out=st_tile[C:C2, stat_i, :],
                                      in_=st_tile[0:C, stat_i, :])
                stat_i += 1
            for (ty, th, col) in tiles:
                b0 = col * C
                nc.sync.dma_start(out=dst[:, ty: ty + th, :],
                                  in_=to[b0: b0 + C, 0:th, :])

        for y0 in range(0, H_out, rows_per_block):
            y1 = min(y0 + rows_per_block, H_out)
            py0 = stride * y0
            py1 = stride * (y1 - 1) + k            # exclusive, padded coords
            iy0, iy1 = py0 - pad, py1 - pad
            v0, v1 = max(iy0, 0), min(iy1, H_in)
            nrows = py1 - py0
            tin = self.iopool.tile([C2, nrows, Wp], dtype,
                                   name=f"{name}_in", tag="conv_in", bufs=2)
            # lower half: zero pads, load, gn+lrelu
            nc.vector.memset(tin[0:C, :, 0:pad], 0.0)
            nc.vector.memset(tin[0:C, :, W_in + pad: Wp], 0.0)
            if v0 > iy0:
                nc.vector.memset(tin[0:C, 0: v0 - iy0, :], 0.0)
            if v1 < iy1:
                nc.vector.memset(tin[0:C, nrows - (iy1 - v1): nrows, :], 0.0)
            nc.sync.dma_start(
                out=tin[0:C, v0 - iy0: v0 - iy0 + (v1 - v0), pad: pad + W_in],
                in_=src[:, v0:v1, :],
            )
            if gn is not None:
                s_t, t_t = gn
                region = tin[0:C, v0 - iy0: v0 - iy0 + (v1 - v0), pad: pad + W_in]
                nc.scalar.activation(out=region, in_=region, func=ACT.Identity,
                                     bias=t_t[0:C, :], scale=s_t[0:C, :])
                if lrelu:
                    nc.vector.scalar_tensor_tensor(
                        out=region, in0=region, scalar=0.2, in1=region,
                        op0=ALU.mult, op1=ALU.max,
                    )
            # upper half = lower shifted left by one column (incl. pads/gn)
            nc.sync.dma_start(out=tin[C:C2, :, 0: Wp - 1], in_=tin[0:C, :, 1:Wp])

            tiles = [(ty, min(rpt, H_out - ty)) for ty in range(y0, y1, rpt)]
            i = 0
            while i < len(tiles):
                if (i + 1 < len(tiles) and tiles[i][1] == rpt
                        and tiles[i + 1][1] == rpt):
                    tyA, tyB = tiles[i][0], tiles[i + 1][0]
                    ps = self.psum.tile([C2, rpt, W_out], F32, name="conv_ps",
                                        tag="conv_ps", bufs=4)
                    emit_tile(tin, py0, tyA, rpt, 0)
                    emit_tile(tin, py0, tyB, rpt, 1)
                    epilogue([(tyA, rpt, 0), (tyB, rpt, 1)])
                    i += 2
                else:
                    ty, th = tiles[i]
                    ps = self.psum.tile([C2, rpt, W_out], F32, name="conv_ps",
                                        tag="conv_ps", bufs=4)
                    emit_tile(tin, py0, ty, th, 0)
                    epilogue([(ty, th, 0)])
                    i += 1
        if stats is not None:
            assert stat_i == stats[1], (stat_i, stats[1])

    # ---- upsample x2 pass (nearest) ---------------------------------------
    def upsample_pass(self, name, src, dst, H, W, dtype, rows_per_block=8):
        nc = self.nc
        for y0 in range(0, H, rows_per_block):
            y1 = min(y0 + rows_per_block, H)
            r = y1 - y0
            tin = self.iopool.tile([C, rows_per_block, W], dtype,
                                   name=f"{name}_uin", tag="u_in", bufs=2)
            nc.sync.dma_start(out=tin[:, 0:r, :], in_=src[:, y0:y1, :])
            tout = self.iopool.tile([C, 2 * rows_per_block, 2 * W], dtype,
                                    name=f"{name}_uout", tag="u_out", bufs=2)
            src_rep = bass.AP(
                tensor=tin.tensor, offset=tin.offset,
                ap=[tin.ap[0], [W, r], [0, 2], [1, W], [0, 2]],
            )
            nc.vector.tensor_copy(
                out=tout[:, 0: 2 * r, :].rearrange(
                    "p (a c) (b d) -> p a c b d", c=2, d=2),
                in_=src_rep,
            )
            nc.sync.dma_start(out=dst[:, 2 * y0: 2 * y1, :],
                              in_=tout[:, 0: 2 * r, :])


def build_net(H=256, W=256, out_name="out", n_cores=N_CORES, dec_dt=BF16):
    _install_patches()
    nc = bass.Bass(target_bir_lowering=False)
    x_ext = nc.declare_dram_parameter("x", [C, H, W], F32, isOutput=False)
    cbta_ext = nc.declare_dram_parameter("cbt_aug", [C + 1, C], F32,
                                         isOutput=False)
    cb_ext = nc.declare_dram_parameter("cb", [C, C], F32, isOutput=False)
    pair_ext = nc.declare_dram_parameter("pairmat2", [C2, C2], F32,
                                         isOutput=False)
    out_ext = nc.declare_dram_parameter(out_name, [C, H, W], F32, isOutput=True)

    tc_cm = TileContext(nc, num_cores=n_cores)
    tc = tc_cm.__enter__()
    import contextlib
    stack = contextlib.ExitStack()
    wpool = stack.enter_context(tc.tile_pool(name="weights", bufs=1))
    iopool = stack.enter_context(tc.tile_pool(name="io", bufs=3))
    smallpool = stack.enter_context(tc.tile_pool(name="small", bufs=1))
    psum = stack.enter_context(tc.tile_pool(name="psum", bufs=4, space="PSUM"))
    psum_small = stack.enter_context(
        tc.tile_pool(name="psum_small", bufs=2, space="PSUM")
    )
    dram = stack.enter_context(tc.tile_pool(name="dram", bufs=1, space="DRAM"))

    net = Net(nc, tc, (wpool, iopool, smallpool, psum, psum_small))
    eps_t = smallpool.tile([C2, 1], F32, name="eps_t", tag="eps_t")
    nc.vector.memset(eps_t[:], EPS)
    net.eps_t = eps_t

    pairmat2 = smallpool.tile([C2, C2], F32, name="pairmat2", tag="pairmat2")
    nc.sync.dma_start(out=pairmat2[:], in_=pair_ext.ap())

    def stats_tile(name, ntiles):
        nt2 = (ntiles + 1) // 2
        t = smallpool.tile([C2, nt2, 6], F32, name=f"stats_{name}", tag="stats",
                           bufs=3)
        return (t, nt2)

    def n_tiles(HH, WW):
        rpt = min(max(1, 512 // WW), HH)
        n_t = (HH + rpt - 1) // rpt
        return (n_t + 1) // 2  # epilogue pairs

    # ---------------- encoder (fp32) ----------------
    H0, W0 = H - 1, W - 1
    H1, W1 = H0 // 2 + 1, W0 // 2 + 1
    H2, W2 = (H1 + 1) // 2, (W1 + 1) // 2

    e0 = dram.tile([C, H0, W0], F32, name="e0", tag="e0")
    e1 = dram.tile([C, H1, W1], F32, name="e1", tag="e1")
    r1 = dram.tile([C, H1, W1], F32, name="r1", tag="r1")
    e2 = dram.tile([C, H1, W1], F32, name="e2", tag="e2")
    e3 = dram.tile([C, H2, W2], F32, name="e3", tag="e3")
    r2 = dram.tile([C, H2, W2], F32, name="r2", tag="r2")
    z = dram.tile([C, H2, W2], F32, name="z", tag="z")

    w_in = net.conv_weights("inconv", 4, F32)
    net.conv_pass("inconv", x_ext.ap(), e0[:], w_in, H, W, H0, W0, 1, F32)

    w_d1 = net.conv_weights("enc1_down", 3, F32)
    st_e1 = stats_tile("e1", n_tiles(H1, W1))
    net.conv_pass("enc1_down", e0[:], e1[:], w_d1, H0, W0, H1, W1, 2, F32,
                  stats=st_e1)
    g1a, b1a = net.gn_weights("enc1_gn1")
    s1a, t1a = net.gn_apply_params("enc1_gn1", st_e1[0], st_e1[1], g1a, b1a,
                                   pairmat2)
    w_c1 = net.conv_weights("enc1_conv1", 3, F32)
    st_r1 = stats_tile("r1", n_tiles(H1, W1))
    net.conv_pass("enc1_conv1", e1[:], r1[:], w_c1, H1, W1, H1, W1, 1, F32,
                  gn=(s1a, t1a), lrelu=True, stats=st_r1)
    g1b, b1b = net.gn_weights("enc1_gn2")
    s1b, t1b = net.gn_apply_params("enc1_gn2", st_r1[0], st_r1[1], g1b, b1b,
                                   pairmat2)
    w_c2 = net.conv_weights("enc1_conv2", 3, F32)
    net.conv_pass("enc1_conv2", r1[:], e2[:], w_c2, H1, W1, H1, W1, 1, F32,
                  gn=(s1b, t1b), lrelu=True, residual=e1[:])

    w_d2 = net.conv_weights("enc2_down", 3, F32)
    st_e3 = stats_tile("e3", n_tiles(H2, W2))
    net.conv_pass("enc2_down", e2[:], e3[:], w_d2, H1, W1, H2, W2, 2, F32,
                  stats=st_e3)
    g2a, b2a = net.gn_weights("enc2_gn1")
    s2a, t2a = net.gn_apply_params("enc2_gn1", st_e3[0], st_e3[1], g2a, b2a,
                                   pairmat2)
    w_c3 = net.conv_weights("enc2_conv1", 3, F32)
    st_r2 = stats_tile("r2", n_tiles(H2, W2))
    net.conv_pass("enc2_conv1", e3[:], r2[:], w_c3, H2, W2, H2, W2, 1, F32,
                  gn=(s2a, t2a), lrelu=True, stats=st_r2)
    g2b, b2b = net.gn_weights("enc2_gn2")
    s2b, t2b = net.gn_apply_params("enc2_gn2", st_r2[0], st_r2[1], g2b, b2b,
                                   pairmat2)
    w_c4 = net.conv_weights("enc2_conv2", 3, F32)
    net.conv_pass("enc2_conv2", r2[:], z[:], w_c4, H2, W2, H2, W2, 1, F32,
                  gn=(s2b, t2b), lrelu=True, residual=e3[:])

    # ---------------- VQ ----------------
    zq = dram.tile([C, H2, W2], dec_dt, name="zq", tag="zq")
    NTOK = H2 * W2
    zbuf = smallpool.tile([C + 1, NTOK], F32, name="zbuf", tag="zbuf")
    nc.vector.memset(zbuf[C: C + 1, :], 1.0)
    nc.sync.dma_start(out=zbuf[0:C, :], in_=z[:].rearrange("c a b -> c (a b)"))
    cbta = smallpool.tile([C + 1, C], F32, name="cbta", tag="cbta")
    nc.sync.dma_start(out=cbta[:], in_=cbta_ext.ap())
    cb_f = smallpool.tile([C, C], F32, name="cb_f", tag="cb_f")
    nc.sync.dma_start(out=cb_f[:], in_=cb_ext.ap())
    cb_bf = smallpool.tile([C, C], dec_dt, name="cb_bf", tag="cb_bf")
    nc.vector.tensor_copy(out=cb_bf[:], in_=cb_f[:])
    ident = smallpool.tile([128, 128], F32, name="ident", tag="ident")
    make_identity(nc, ident[:])

    TTOK = min(128, NTOK)
    for tt in range(0, NTOK, TTOK):
        sc_ps = psum_small.tile([TTOK, C], F32, name="vq_s", tag="vq_ps", bufs=3)
        nc.tensor.matmul(sc_ps[:], zbuf[:, tt: tt + TTOK], cbta[:],
                         start=True, stop=True)
        mx = smallpool.tile([TTOK, 1], F32, name="vq_mx", tag="vq_mx", bufs=2)
        nc.vector.tensor_reduce(out=mx[:], in_=sc_ps[:], axis=AX.X, op=ALU.max)
        oh = smallpool.tile([TTOK, C], F32, name="vq_oh", tag="vq_oh", bufs=2)
        nc.vector.tensor_scalar(
            out=oh[:], in0=sc_ps[:], scalar1=mx[:], scalar2=None, op0=ALU.is_ge
        )
        oht_ps = psum_small.tile([C, TTOK], F32, name="vq_oht", tag="vq_ps",
                                 bufs=3)
        nc.tensor.transpose(oht_ps[:], oh[:], ident[0:TTOK, 0:TTOK])
        oht = smallpool.tile([C, TTOK], dec_dt, name="vq_ohtb", tag="vq_ohtb",
                             bufs=2)
        nc.vector.tensor_copy(out=oht[:], in_=oht_ps[:])
        zq_ps = psum_small.tile([C, TTOK], F32, name="vq_zq", tag="vq_ps", bufs=3)
        nc.tensor.matmul(zq_ps[:], cb_bf[:], oht[:], start=True, stop=True)
        zq_sb = smallpool.tile([C, TTOK], dec_dt, name="vq_zqsb", tag="vq_zqsb",
                               bufs=2)
        nc.vector.tensor_copy(out=zq_sb[:], in_=zq_ps[:])
        nc.sync.dma_start(
            out=zq[:].rearrange("c a b -> c (a b)")[:, tt: tt + TTOK],
            in_=zq_sb[:],
        )

    # ---------------- decoder ----------------
    up1 = dram.tile([C, H1, W1], dec_dt, name="up1", tag="up1")
    d0 = dram.tile([C, H1, W1], dec_dt, name="d0", tag="d0")
    rA = dram.tile([C, H1, W1], dec_dt, name="rA", tag="rA")
    d1 = dram.tile([C, H1, W1], dec_dt, name="d1", tag="d1")
    rB = dram.tile([C, H1, W1], dec_dt, name="rB", tag="rB")
    d2 = dram.tile([C, H1, W1], dec_dt, name="d2", tag="d2")
    up2 = dram.tile([C, H, W], dec_dt, name="up2", tag="up2")
    D0 = dram.tile([C, H, W], dec_dt, name="D0", tag="D0")
    RA = dram.tile([C, H, W], dec_dt, name="RA", tag="RA")
    D1 = dram.tile([C, H, W], dec_dt, name="D1", tag="D1")
    RB = dram.tile([C, H, W], dec_dt, name="RB", tag="RB")

    net.upsample_pass("up1", zq[:], up1[:], H2, W2, dec_dt)

    def dec_block(pref, up_src, conv_out, r_out, mid_out, r2_out, final_out,
                  HH, WW, final_dtype, final_stats=None):
        n_t = n_tiles(HH, WW)
        w_u = net.conv_weights(f"{pref}_conv", 3, dec_dt)
        st0 = stats_tile(f"{pref}_c", n_t)
        net.conv_pass(f"{pref}_conv", up_src, conv_out, w_u,
                      HH, WW, HH, WW, 1, dec_dt, stats=st0)
        g_a, bt_a = net.gn_weights(f"{pref}_r1gn1")
        s_a, t_a = net.gn_apply_params(f"{pref}_r1gn1", st0[0], st0[1], g_a,
                                       bt_a, pairmat2)
        w_1 = net.conv_weights(f"{pref}_r1c1", 3, dec_dt)
        st1 = stats_tile(f"{pref}_r1", n_t)
        net.conv_pass(f"{pref}_r1c1", conv_out, r_out, w_1,
                      HH, WW, HH, WW, 1, dec_dt, gn=(s_a, t_a), lrelu=True,
                      stats=st1)
        g_b, bt_b = net.gn_weights(f"{pref}_r1gn2")
        s_b, t_b = net.gn_apply_params(f"{pref}_r1gn2", st1[0], st1[1], g_b,
                                       bt_b, pairmat2)
        w_2 = net.conv_weights(f"{pref}_r1c2", 3, dec_dt)
        st2 = stats_tile(f"{pref}_m", n_t)
        net.conv_pass(f"{pref}_r1c2", r_out, mid_out, w_2,
                      HH, WW, HH, WW, 1, dec_dt, gn=(s_b, t_b), lrelu=True,
                      residual=conv_out, stats=st2)
        g_c, bt_c = net.gn_weights(f"{pref}_r2gn1")
        s_c, t_c = net.gn_apply_params(f"{pref}_r2gn1", st2[0], st2[1], g_c,
                                       bt_c, pairmat2)
        w_3 = net.conv_weights(f"{pref}_r2c1", 3, dec_dt)
        st3 = stats_tile(f"{pref}_r2", n_t)
        net.conv_pass(f"{pref}_r2c1", mid_out, r2_out, w_3,
                      HH, WW, HH, WW, 1, dec_dt, gn=(s_c, t_c), lrelu=True,
                      stats=st3)
        g_d, bt_d = net.gn_weights(f"{pref}_r2gn2")
        s_d, t_d = net.gn_apply_params(f"{pref}_r2gn2", st3[0], st3[1], g_d,
                                       bt_d, pairmat2)
        w_4 = net.conv_weights(f"{pref}_r2c2", 3, dec_dt)
        net.conv_pass(f"{pref}_r2c2", r2_out, final_out, w_4,
                      HH, WW, HH, WW, 1, dec_dt, gn=(s_d, t_d), lrelu=True,
                      residual=mid_out, stats=final_stats,
                      out_dtype=final_dtype)

    dec_block("dec1", up1[:], d0[:], rA[:], d1[:], rB[:], d2[:],
              H1, W1, dec_dt, final_stats=None)
    net.upsample_pass("up2", d2[:], up2[:], H1, W1, dec_dt)
    dec_block("dec2", up2[:], D0[:], RA[:], D1[:], RB[:], out_ext.ap(),
              H, W, F32, final_stats=None)

    stack.close()
    tc_cm.__exit__(None, None, None)
    return nc


# ---------------------------------------------------------------------------
# Host side
# ---------------------------------------------------------------------------

def _pack_conv_pair(w):
    """[co, ci, k, k] -> (wp [128, npair, co], ws [64, ns, co] or None)"""
    co, ci, kh, kw = w.shape
    k = kh
    wt = np.transpose(w, (1, 2, 3, 0)).astype(np.float32)  # [ci, ky, kx, co]
    pairs, singles = _taps_layout(k)
    wp = np.zeros((2 * ci, len(pairs), co), np.float32)
    for i, (ky, kx) in enumerate(pairs):
        wp[0:ci, i, :] = wt[:, ky, kx, :]
        wp[ci:, i, :] = wt[:, ky, kx + 1, :]
    ws = None
    if singles:
        ws = np.zeros((ci, len(singles), co), np.float32)
        for i, (ky, kx) in enumerate(singles):
            ws[:, i, :] = wt[:, ky, kx, :]
    return np.ascontiguousarray(wp), (
        np.ascontiguousarray(ws) if ws is not None else None
    )


def _flatten_params(params):
    out = {}

    def conv(name, cp):
        wp, ws = _pack_conv_pair(np.asarray(cp["w"]))
        out[f"{name}_wp"] = wp
        if ws is not None:
            out[f"{name}_ws"] = ws
        out[f"{name}_b"] = np.asarray(cp["b"], dtype=np.float32)

    def gn(name, gp):
        out[f"{name}_g"] = np.asarray(gp["g"], dtype=np.float32)
        out[f"{name}_bt"] = np.asarray(gp["b"], dtype=np.float32)

    conv("inconv", params["in_conv"])
    enc = params["enc"]
    conv("enc1_down", enc[0]["down"])
    gn("enc1_gn1", enc[0]["res"]["gn1"])
    conv("enc1_conv1", enc[0]["res"]["conv1"])
    gn("enc1_gn2", enc[0]["res"]["gn2"])
    conv("enc1_conv2", enc[0]["res"]["conv2"])
    conv("enc2_down", enc[1]["down"])
    gn("enc2_gn1", enc[1]["res"]["gn1"])
    conv("enc2_conv1", enc[1]["res"]["conv1"])
    gn("enc2_gn2", enc[1]["res"]["gn2"])
    conv("enc2_conv2", enc[1]["res"]["conv2"])

    cb = np.asarray(params["codebook"], dtype=np.float32)
    out["cb"] = cb
    out["cbt_aug"] = np.concatenate(
        [cb.T, (-0.5 * (cb * cb).sum(1))[None, :]], axis=0
    ).astype(np.float32)
    per = C // GROUPS
    pm = np.zeros((C2, C2), np.float32)
    for c2 in range(C2):
        c = c2 % C
        g0 = (c // per) * per
        for rep in range(2):
            pm[c2, rep * C + g0: rep * C + g0 + per] = 1.0 / (2 * per)
    # lhsT convention: out[m] = sum_k lhsT[k, m] * rhs[k] -> build transpose
    out["pairmat2"] = np.ascontiguousarray(pm.T)

    dec = params["dec"]
    for i, pref in enumerate(["dec1", "dec2"]):
        blk = dec[i]
        conv(f"{pref}_conv", blk["conv"])
        gn(f"{pref}_r1gn1", blk["res1"]["gn1"])
        conv(f"{pref}_r1c1", blk["res1"]["conv1"])
        gn(f"{pref}_r1gn2", blk["res1"]["gn2"])
        conv(f"{pref}_r1c2", blk["res1"]["conv2"])
        gn(f"{pref}_r2gn1", blk["res2"]["gn1"])
        conv(f"{pref}_r2c1", blk["res2"]["conv1"])
        gn(f"{pref}_r2gn2", blk["res2"]["gn2"])
        conv(f"{pref}_r2c2", blk["res2"]["conv2"])
    return out


def _make_in_maps(x, params):
    p = _flatten_params(params)
    maps = []
    for i in range(N_CORES):
        m = dict(p)
        m["x"] = np.ascontiguousarray(np.asarray(x)[i], dtype=np.float32)
        maps.append(m)
    return maps


_CACHED = {}


def _ensure_ntff_hook():
    try:
        from antenv.axon_hooks import get_axon_ntff_profile_hook  # noqa: F401
        return
    except ImportError:
        pass
    try:
        import types
        import antenv
        from trn_agent_boot.trn_boot import _ntff_profile_via_ctypes

        hook = _ntff_profile_via_ctypes("/opt/axon/libaxon_pjrt.so")
        mod = types.ModuleType("antenv.axon_hooks")
        _h = [hook]
        mod.set_axon_ntff_profile_hook = lambda h: _h.__setitem__(0, h)
        mod.get_axon_ntff_profile_hook = lambda: _h[0]
        sys.modules["antenv.axon_hooks"] = mod
        antenv.axon_hooks = mod
    except Exception as e:  # pragma: no cover
        print("ntff hook install failed:", e)


def _run(x, params, trace=False):
    if trace:
        _ensure_ntff_hook()
    x = np.asarray(x)
    key = "net"
    if key not in _CACHED:
        _CACHED[key] = build_net()
    nc = _CACHED[key]
    in_maps = _make_in_maps(x, params)
    res = run_bass_kernel_spmd(
        nc, in_maps, core_ids=list(range(N_CORES)), trace=trace
    )
    out = np.stack([res.results[i]["out"] for i in range(N_CORES)], axis=0)
    return out, res.exec_time_ns


def kernel(x, params):
    return _run(x, params, trace=False)[0]


def kernel_timed(x, params):
    return _run(x, params, trace=True)


# revision 15
# speedup vs baseline: 1.8746x; 1.8746x over previous
"""Trainium2 Bass kernel for the VQ-VAE-ish reference net (vq_codebook).

Data-parallel: 8 images -> 8 NeuronCores, one image per core. Inside each
core the full encoder -> VQ -> decoder chain runs as tap-matmul convolutions
(channels on partitions, weights stationary, shifted-image APs as the moving
operand), with GroupNorm stats fused into the producing conv's epilogue.

PE-array packing: the input tile is replicated into partitions 64..127
shifted by one column (SBUF->SBUF DMA), so two horizontal taps contract in a
single K=128 matmul; two output row-tiles run concurrently in the left/right
array halves via tile_position=(0,0)/(0,64). k=4 (in_conv) packs perfectly
(8 pair-matmuls); k=3 packs 9 taps into 6 matmul slots.

Precision: encoder + VQ in fp32 (argmin flips are catastrophic: 1 flip costs
~1e-2 output rel err), decoder in bf16 (measured ~7e-3 rel err).
"""
import sys

for _p in ("/opt/trn_rl_repo", "/root/.axon_site/_ro/trn_rl_repo"):
    if _p not in sys.path:
        sys.path.insert(0, _p)

import numpy as np
import concourse.bass as bass
import concourse.mybir as mybir
from concourse.bass_utils import run_bass_kernel_spmd
from concourse.tile import TileContext
from concourse.vector_clock import ScopedClock
from concourse.bass_primitives_rust import SemaphoreHandle
from concourse.masks import make_identity

F32 = mybir.dt.float32
BF16 = mybir.dt.bfloat16
AX = mybir.AxisListType
ALU = mybir.AluOpType
ACT = mybir.ActivationFunctionType

C = 64
C2 = 128
GROUPS = 32
EPS = 1e-6
N_CORES = 8

# ---------------------------------------------------------------------------
# walrus in this container rejects >1 sync wait per instruction: split excess
# waits into standalone InstEventSemaphore instructions after Tile scheduling.
# ---------------------------------------------------------------------------
WAIT_LIMIT = 1
_split_counter = [0]


def _split_excess_waits(nc, limit=WAIT_LIMIT):
    f = nc.m.functions[0]
    for bb in f.blocks:
        insts = bb.instructions
        out = []
        changed = False
        for inst in insts:
            si = inst.sync_info
            if si is not None and len(si.on_wait) > limit:
                waits = list(si.on_wait)
                extra, keep = waits[: len(waits) - limit], waits[len(waits) - limit:]
                for w in extra:
                    _split_counter[0] += 1
                    nop = mybir.InstEventSemaphore(
                        name=f"I-waitsplit-{_split_counter[0]}", ins=[], outs=[]
                    )
                    nop.engine = inst.engine
                    nop.sync_info = mybir.SyncInfo(on_wait=[w], on_update=[])
                    nc.register_instruction(nop, overwrite=True)
                    out.append(nop)
                inst.sync_info = mybir.SyncInfo(
                    on_wait=keep, on_update=list(si.on_update)
                )
                changed = True
            out.append(inst)
        if changed:
            bb.instructions = out


def _patched_drain_and_barrier(self, tick_clock, wait_clock):
    nc = self.nc
    drain_inst = nc.sync.drain()
    wait_clock.add_sem_waits(
        drain_inst.ins, ScopedClock({None: tick_clock.global_clock})
    )
    si = drain_inst.ins.sync_info
    waits = list(si.on_wait) if si is not None else []
    if len(waits) > 1:
        drain_inst.ins.sync_info = mybir.SyncInfo(
            on_wait=[], on_update=list(si.on_update)
        )
        for w in waits:
            nc.sync.wait_ge(SemaphoreHandle(w.ant_name, w.id), w.wait_value)
        nc.sync.drain()
    nc.all_engine_barrier()
    popped = nc._tile_sem_poison_stack.pop()
    assert popped is self._sem_poison
    nc.clear_and_free_semaphores(list(self.sems.allocated().values()))
    nc.all_engine_barrier()


def _install_patches():
    if getattr(TileContext, "_vqnet_patched", False):
        return
    TileContext._drain_and_barrier = _patched_drain_and_barrier
    _orig_exit = TileContext.__exit__

    def _patched_exit(self, exc_type, exc_val, exc_tb):
        r = _orig_exit(self, exc_type, exc_val, exc_tb)
        if exc_type is None:
            _split_excess_waits(self.nc)
        return r

    TileContext.__exit__ = _patched_exit
    TileContext._vqnet_patched = True


def _taps_layout(k):
    """pairs: [(ky, kx)] covering (ky,kx)+(ky,kx+1); singles: [(ky, k-1)]."""
    pairs = [(ky, kx) for ky in range(k) for kx in range(0, k - 1, 2)]
    singles = [(ky, k - 1) for ky in range(k)] if k % 2 == 1 else []
    return pairs, singles


# ---------------------------------------------------------------------------
# Builder
# ---------------------------------------------------------------------------

class Net:
    def __init__(self, nc, tc, pools):
        self.nc = nc
        self.tc = tc
        self.wpool, self.iopool, self.smallpool, self.psum, self.psum_small = pools

    # ---- weights -----------------------------------------------------------
    def conv_weights(self, name, k, dtype):
        """Packed weights: wp [C2, npair, C], ws [C, ns, C] (ns may be 0),
        bias duplicated to [C2, 1]."""
        nc = self.nc
        pairs, singles = _taps_layout(k)
        wp_ext = nc.declare_dram_parameter(
            f"{name}_wp", [C2, len(pairs), C], F32, isOutput=False
        )
        wp_s = self.wpool.tile([C2, len(pairs), C], dtype,
                               name=f"{name}_wp", tag=f"{name}_wp")
        if dtype == F32:
            nc.sync.dma_start(out=wp_s[:], in_=wp_ext.ap())
        else:
            wp_f = self.iopool.tile([C2, len(pairs), C], F32,
                                    name=f"{name}_wpf", tag="wstage", bufs=2)
            nc.sync.dma_start(out=wp_f[:], in_=wp_ext.ap())
            nc.vector.tensor_copy(out=wp_s[:], in_=wp_f[:])
        ws_s = None
        if singles:
            ws_ext = nc.declare_dram_parameter(
                f"{name}_ws", [C, len(singles), C], F32, isOutput=False
            )
            ws_s = self.wpool.tile([C, len(singles), C], dtype,
                                   name=f"{name}_ws", tag=f"{name}_ws")
            if dtype == F32:
                nc.sync.dma_start(out=ws_s[:], in_=ws_ext.ap())
            else:
                ws_f = self.iopool.tile([C, len(singles), C], F32,
                                        name=f"{name}_wsf", tag="wstage", bufs=2)
                nc.sync.dma_start(out=ws_f[:], in_=ws_ext.ap())
                nc.vector.tensor_copy(out=ws_s[:], in_=ws_f[:])
        b_ext = nc.declare_dram_parameter(f"{name}_b", [C], F32, isOutput=False)
        b_s = self.wpool.tile([C2, 1], F32, name=f"{name}_b", tag=f"{name}_b")
        nc.sync.dma_start(out=b_s[0:C, :], in_=b_ext.ap().unsqueeze(-1))
        nc.sync.dma_start(out=b_s[C:C2, :], in_=b_ext.ap().unsqueeze(-1))
        return (wp_s, ws_s, b_s, k)

    def gn_weights(self, name):
        nc = self.nc
        g_ext = nc.declare_dram_parameter(f"{name}_g", [C], F32, isOutput=False)
        b_ext = nc.declare_dram_parameter(f"{name}_bt", [C], F32, isOutput=False)
        g_s = self.wpool.tile([C2, 1], F32, name=f"{name}_g", tag=f"{name}_g")
        b_s = self.wpool.tile([C2, 1], F32, name=f"{name}_bt", tag=f"{name}_bt")
        nc.sync.dma_start(out=g_s[0:C, :], in_=g_ext.ap().unsqueeze(-1))
        nc.sync.dma_start(out=g_s[C:C2, :], in_=g_ext.ap().unsqueeze(-1))
        nc.sync.dma_start(out=b_s[0:C, :], in_=b_ext.ap().unsqueeze(-1))
        nc.sync.dma_start(out=b_s[C:C2, :], in_=b_ext.ap().unsqueeze(-1))
        return g_s, b_s

    # ---- GroupNorm scale/bias derivation ----------------------------------
    def gn_apply_params(self, name, stats, nstat, gamma, beta, pairmat2):
        """stats [C2, nstat, 6] -> s,t [C2,1] with y = s*x + t."""
        nc = self.nc
        sp = self.smallpool
        mv = sp.tile([C2, 2], F32, name=f"{name}_mv", tag=f"{name}_mv")
        nc.vector.bn_aggr(out=mv[:], in_=stats[:, 0:nstat, :])
        pk = sp.tile([C2, 2], F32, name=f"{name}_pk", tag=f"{name}_pk")
        nc.vector.tensor_copy(out=pk[:, 0:1], in_=mv[:, 0:1])
        nc.vector.tensor_tensor(
            out=pk[:, 1:2], in0=mv[:, 0:1], in1=mv[:, 0:1], op=ALU.mult
        )
        nc.vector.tensor_tensor(
            out=pk[:, 1:2], in0=pk[:, 1:2], in1=mv[:, 1:2], op=ALU.add
        )
        psm = self.psum_small.tile([C2, 2], F32, name="gn_psum", tag="gn_psum",
                                   bufs=1)
        nc.tensor.matmul(psm[:], pairmat2[:], pk[:], start=True, stop=True)
        gmean = sp.tile([C2, 1], F32, name=f"{name}_gmean", tag=f"{name}_gmean")
        nc.vector.tensor_copy(out=gmean[:], in_=psm[:, 0:1])
        varg = sp.tile([C2, 1], F32, name=f"{name}_varg", tag=f"{name}_varg")
        nc.vector.tensor_tensor(out=varg[:], in0=gmean[:], in1=gmean[:],
                                op=ALU.mult)
        nc.vector.tensor_tensor(out=varg[:], in0=psm[:, 1:2], in1=varg[:],
                                op=ALU.subtract)
        nc.scalar.activation(out=varg[:], in_=varg[:], func=ACT.Sqrt,
                             bias=self.eps_t[:], scale=1.0)
        nc.vector.reciprocal(out=varg[:], in_=varg[:])
        s_t = sp.tile([C2, 1], F32, name=f"{name}_s", tag=f"{name}_s")
        t_t = sp.tile([C2, 1], F32, name=f"{name}_t", tag=f"{name}_t")
        nc.vector.tensor_tensor(out=s_t[:], in0=gamma[:], in1=varg[:], op=ALU.mult)
        nc.vector.tensor_tensor(out=t_t[:], in0=gmean[:], in1=s_t[:], op=ALU.mult)
        nc.vector.tensor_tensor(out=t_t[:], in0=beta[:], in1=t_t[:],
                                op=ALU.subtract)
        return s_t, t_t

    # ---- packed conv pass --------------------------------------------------
    def conv_pass(
        self, name, src, dst, wpack,
        H_in, W_in, H_out, W_out, stride, dtype,
        gn=None, lrelu=False, residual=None, stats=None,
        rows_per_block=16, out_dtype=None,
    ):
        nc = self.nc
        wp_s, ws_s, b_s, k = wpack
        pairs, singles = _taps_layout(k)
        pad = 1
        Wp = W_in + 2 * pad
        out_dtype = out_dtype or dtype
        rpt = min(max(1, 512 // W_out), H_out)   # rows per psum tile
        assert rpt * W_out <= 512
        rows_per_block = max(2 * rpt, (rows_per_block // rpt) * rpt)
        stat_i = 0
        n_slots = len(pairs) + len(singles)

        def emit_tile(tin, py0, ty, th, col):
            """Matmuls for one output row-tile into psum column half `col`."""
            base = col * C
            slot = 0
            for (ky, kx) in pairs + singles:
                is_pair = slot < len(pairs)
                r0 = stride * ty - py0 + ky
                npart = C2 if is_pair else C
                rhs = bass.AP(
                    tensor=tin.tensor,
                    offset=tin.offset + r0 * Wp + kx,
                    ap=[[tin.ap[0][0], npart], [stride * Wp, th],
                        [stride, W_out]],
                )
                lhsT = wp_s[:, slot, :] if is_pair else ws_s[:, slot - len(pairs), :]
                nc.tensor.matmul(
                    ps[base: base + C, 0:th, :], lhsT, rhs,
                    start=(slot == 0), stop=(slot == n_slots - 1),
                    tile_position=(0, base),
                    skip_group_check=True,
                )
                slot += 1

        def epilogue(tiles):
            """tiles: list of (ty, th, col). Emits bias/residual/stats/DMA."""
            nonlocal stat_i
            np_ = len(tiles) * C
            to = self.iopool.tile([C2, rpt, W_out], out_dtype,
                                  name=f"{name}_out", tag="conv_out", bufs=3)
            lo = tiles[0][2] * C
            hi = lo + np_
            assert lo == 0
            full = to[0:np_, 0: tiles[0][1], :]
            nc.scalar.activation(
                out=full, in_=ps[0:np_, 0: tiles[0][1], :],
                func=ACT.Identity, bias=b_s[0:np_, :], scale=1.0,
            ) if all(t[1] == tiles[0][1] for t in tiles) else None
            if not all(t[1] == tiles[0][1] for t in tiles):
                for (ty, th, col) in tiles:
                    b0 = col * C
                    nc.scalar.activation(
                        out=to[b0: b0 + C, 0:th, :], in_=ps[b0: b0 + C, 0:th, :],
                        func=ACT.Identity, bias=b_s[b0: b0 + C, :], scale=1.0,
                    )
            if residual is not None:
                tr = self.iopool.tile([C2, rpt, W_out], dtype,
                                      name=f"{name}_res", tag="conv_res", bufs=3)
                for (ty, th, col) in tiles:
                    b0 = col * C
                    nc.sync.dma_start(out=tr[b0: b0 + C, 0:th, :],
                                      in_=residual[:, ty: ty + th, :])
                    nc.vector.tensor_tensor(
                        out=to[b0: b0 + C, 0:th, :], in0=to[b0: b0 + C, 0:th, :],
                        in1=tr[b0: b0 + C, 0:th, :], op=ALU.add,
                    )
            if stats is not None:
                st_tile, _ = stats
                if len(tiles) == 2:
                    nc.vector.bn_stats(
                        out=st_tile[:, stat_i, :],
                        in_=to[:, 0: tiles[0][1], :].rearrange(
                            "p a b -> p (a b)"),
                    )
                else:
                    (ty, th, col) = tiles[0]
                    nc.vector.bn_stats(
                        out=st_tile[0:C, stat_i, :],
                        in_=to[0:C, 0:th, :].rearrange("p a b -> p (a b)"),
                    )
                    # duplicate lower-half stats into upper half (tiny DMA)
                    nc.sync.dma_start(out=st_tile[C:C2, stat_i, :],
                                      in_=st_tile[0:C, stat_i, :])
                stat_i += 1
            for (ty, th, col) in tiles:
                b0 = col * C
                nc.sync.dma_start(out=dst[:, ty: ty + th, :],
                                  in_=to[b0: b0 + C, 0:th, :])

        for y0 in range(0, H_out, rows_per_block):
            y1 = min(y0 + rows_per_block, H_out)
            py0 = stride * y0
            py1 = stride * (y1 - 1) + k            # exclusive, padded coords
            iy0, iy1 = py0 - pad, py1 - pad
            v0, v1 = max(iy0, 0), min(iy1, H_in)
            nrows = py1 - py0
            tin = self.iopool.tile([C2, nrows, Wp], dtype,
                                   name=f"{name}_in", tag="conv_in", bufs=2)
            # lower half: zero pads, load, gn+lrelu
            nc.vector.memset(tin[0:C, :, 0:pad], 0.0)
            nc.vector.memset(tin[0:C, :, W_in + pad: Wp], 0.0)
            if v0 > iy0:
                nc.vector.memset(tin[0:C, 0: v0 - iy0, :], 0.0)
            if v1 < iy1:
                nc.vector.memset(tin[0:C, nrows - (iy1 - v1): nrows, :], 0.0)
            nc.sync.dma_start(
                out=tin[0:C, v0 - iy0: v0 - iy0 + (v1 - v0), pad: pad + W_in],
                in_=src[:, v0:v1, :],
            )
            if gn is not None:
                s_t, t_t = gn
                region = tin[0:C, v0 - iy0: v0 - iy0 + (v1 - v0), pad: pad + W_in]
                nc.scalar.activation(out=region, in_=region, func=ACT.Identity,
                                     bias=t_t[0:C, :], scale=s_t[0:C, :])
                if lrelu:
                    nc.vector.scalar_tensor_tensor(
                        out=region, in0=region, scalar=0.2, in1=region,
                        op0=ALU.mult, op1=ALU.max,
                    )
            # upper half = lower shifted left by one column (incl. pads/gn)
            nc.sync.dma_start(out=tin[C:C2, :, 0: Wp - 1], in_=tin[0:C, :, 1:Wp])

            tiles = [(ty, min(rpt, H_out - ty)) for ty in range(y0, y1, rpt)]
            i = 0
            while i < len(tiles):
                if (i + 1 < len(tiles) and tiles[i][1] == rpt
                        and tiles[i + 1][1] == rpt):
                    tyA, tyB = tiles[i][0], tiles[i + 1][0]
                    ps = self.psum.tile([C2, rpt, W_out], F32, name="conv_ps",
                                        tag="conv_ps", bufs=4)
                    emit_tile(tin, py0, tyA, rpt, 0)
                    emit_tile(tin, py0, tyB, rpt, 1)
                    epilogue([(tyA, rpt, 0), (tyB, rpt, 1)])
                    i += 2
                else:
                    ty, th = tiles[i]
                    ps = self.psum.tile([C2, rpt, W_out], F32, name="conv_ps",
                                        tag="conv_ps", bufs=4)
                    emit_tile(tin, py0, ty, th, 0)
                    epilogue([(ty, th, 0)])
                    i += 1
        if stats is not None:
            assert stat_i == stats[1], (stat_i, stats[1])

    # ---- upsample x2 pass (nearest) ---------------------------------------
    def upsample_pass(self, name, src, dst, H, W, dtype, rows_per_block=8):
        nc = self.nc
        for y0 in range(0, H, rows_per_block):
            y1 = min(y0 + rows_per_block, H)
            r = y1 - y0
            tin = self.iopool.tile([C, rows_per_block, W], dtype,
                                   name=f"{name}_uin", tag="u_in", bufs=2)
            nc.sync.dma_start(out=tin[:, 0:r, :], in_=src[:, y0:y1, :])
            tout = self.iopool.tile([C, 2 * rows_per_block, 2 * W], dtype,
                                    name=f"{name}_uout", tag="u_out", bufs=2)
            src_rep = bass.AP(
                tensor=tin.tensor, offset=tin.offset,
                ap=[tin.ap[0], [W, r], [0, 2], [1, W], [0, 2]],
            )
            nc.vector.tensor_copy(
                out=tout[:, 0: 2 * r, :].rearrange(
                    "p (a c) (b d) -> p a c b d", c=2, d=2),
                in_=src_rep,
            )
            nc.sync.dma_start(out=dst[:, 2 * y0: 2 * y1, :],
                              in_=tout[:, 0: 2 * r, :])


def build_net(H=256, W=256, out_name="out", n_cores=N_CORES, dec_dt=BF16, stop_after=None):
    _install_patches()
    nc = bass.Bass(target_bir_lowering=False)
    x_ext = nc.declare_dram_parameter("x", [C, H, W], F32, isOutput=False)
    cbt_ext = nc.declare_dram_parameter("cbt", [C, C], F32, isOutput=False)
    cbn_ext = nc.declare_dram_parameter("cbn", [C], F32, isOutput=False)
    vqrow_ext = nc.declare_dram_parameter("vqrow", [C], F32, isOutput=False)
    cb_ext = nc.declare_dram_parameter("cb", [C, C], F32, isOutput=False)
    pair_ext = nc.declare_dram_parameter("pairmat2", [C2, C2], F32,
                                         isOutput=False)
    out_ext = nc.declare_dram_parameter(out_name, [C, H, W], F32, isOutput=True)

    tc_cm = TileContext(nc, num_cores=n_cores)
    tc = tc_cm.__enter__()
    import contextlib
    stack = contextlib.ExitStack()
    wpool = stack.enter_context(tc.tile_pool(name="weights", bufs=1))
    iopool = stack.enter_context(tc.tile_pool(name="io", bufs=3))
    smallpool = stack.enter_context(tc.tile_pool(name="small", bufs=1))
    psum = stack.enter_context(tc.tile_pool(name="psum", bufs=4, space="PSUM"))
    psum_small = stack.enter_context(
        tc.tile_pool(name="psum_small", bufs=2, space="PSUM")
    )
    dram = stack.enter_context(tc.tile_pool(name="dram", bufs=1, space="DRAM"))

    net = Net(nc, tc, (wpool, iopool, smallpool, psum, psum_small))
    eps_t = smallpool.tile([C2, 1], F32, name="eps_t", tag="eps_t")
    nc.vector.memset(eps_t[:], EPS)
    net.eps_t = eps_t

    pairmat2 = smallpool.tile([C2, C2], F32, name="pairmat2", tag="pairmat2")
    nc.sync.dma_start(out=pairmat2[:], in_=pair_ext.ap())

    def stats_tile(name, ntiles):
        t = smallpool.tile([C2, ntiles, 6], F32, name=f"stats_{name}",
                           tag="stats", bufs=3)
        return (t, ntiles)

    def n_tiles(HH, WW, rows_per_block=16):
        # must mirror conv_pass's block/pair loop exactly
        rpt = min(max(1, 512 // WW), HH)
        rpb = max(2 * rpt, (rows_per_block // rpt) * rpt)
        n = 0
        for y0 in range(0, HH, rpb):
            y1 = min(y0 + rpb, HH)
            tiles = [(ty, min(rpt, HH - ty)) for ty in range(y0, y1, rpt)]
            i = 0
            while i < len(tiles):
                if (i + 1 < len(tiles) and tiles[i][1] == rpt
                        and tiles[i + 1][1] == rpt):
                    i += 2
                else:
                    i += 1
                n += 1
        return n

    # ---------------- encoder (fp32) ----------------
    H0, W0 = H - 1, W - 1
    H1, W1 = H0 // 2 + 1, W0 // 2 + 1
    H2, W2 = (H1 + 1) // 2, (W1 + 1) // 2

    e0 = dram.tile([C, H0, W0], F32, name="e0", tag="e0")
    e1 = dram.tile([C, H1, W1], F32, name="e1", tag="e1")
    r1 = dram.tile([C, H1, W1], F32, name="r1", tag="r1")
    e2 = dram.tile([C, H1, W1], F32, name="e2", tag="e2")
    e3 = dram.tile([C, H2, W2], F32, name="e3", tag="e3")
    r2 = dram.tile([C, H2, W2], F32, name="r2", tag="r2")
    z = dram.tile([C, H2, W2], F32, name="z", tag="z")

    w_in = net.conv_weights("inconv", 4, F32)
    net.conv_pass("inconv", x_ext.ap(), e0[:], w_in, H, W, H0, W0, 1, F32)

    w_d1 = net.conv_weights("enc1_down", 3, F32)
    st_e1 = stats_tile("e1", n_tiles(H1, W1))
    net.conv_pass("enc1_down", e0[:], e1[:], w_d1, H0, W0, H1, W1, 2, F32,
                  stats=st_e1)
    g1a, b1a = net.gn_weights("enc1_gn1")
    s1a, t1a = net.gn_apply_params("enc1_gn1", st_e1[0], st_e1[1], g1a, b1a,
                                   pairmat2)
    w_c1 = net.conv_weights("enc1_conv1", 3, F32)
    st_r1 = stats_tile("r1", n_tiles(H1, W1))
    net.conv_pass("enc1_conv1", e1[:], r1[:], w_c1, H1, W1, H1, W1, 1, F32,
                  gn=(s1a, t1a), lrelu=True, stats=st_r1)
    g1b, b1b = net.gn_weights("enc1_gn2")
    s1b, t1b = net.gn_apply_params("enc1_gn2", st_r1[0], st_r1[1], g1b, b1b,
                                   pairmat2)
    w_c2 = net.conv_weights("enc1_conv2", 3, F32)
    net.conv_pass("enc1_conv2", r1[:], e2[:], w_c2, H1, W1, H1, W1, 1, F32,
                  gn=(s1b, t1b), lrelu=True, residual=e1[:])

    w_d2 = net.conv_weights("enc2_down", 3, F32)
    st_e3 = stats_tile("e3", n_tiles(H2, W2))
    net.conv_pass("enc2_down", e2[:], e3[:], w_d2, H1, W1, H2, W2, 2, F32,
                  stats=st_e3)
    g2a, b2a = net.gn_weights("enc2_gn1")
    s2a, t2a = net.gn_apply_params("enc2_gn1", st_e3[0], st_e3[1], g2a, b2a,
                                   pairmat2)
    w_c3 = net.conv_weights("enc2_conv1", 3, F32)
    st_r2 = stats_tile("r2", n_tiles(H2, W2))
    net.conv_pass("enc2_conv1", e3[:], r2[:], w_c3, H2, W2, H2, W2, 1, F32,
                  gn=(s2a, t2a), lrelu=True, stats=st_r2)
    g2b, b2b = net.gn_weights("enc2_gn2")
    s2b, t2b = net.gn_apply_params("enc2_gn2", st_r2[0], st_r2[1], g2b, b2b,
                                   pairmat2)
    w_c4 = net.conv_weights("enc2_conv2", 3, F32)
    net.conv_pass("enc2_conv2", r2[:], z[:], w_c4, H2, W2, H2, W2, 1, F32,
                  gn=(s2b, t2b), lrelu=True, residual=e3[:])

    if stop_after == "enc":
        to = smallpool.tile([C, H2 * W2], F32, name="dbg_enc", tag="dbg")
        nc.sync.dma_start(out=to[:], in_=z[:].rearrange("c a b -> c (a b)"))
        nc.sync.dma_start(
            out=out_ext.ap().rearrange("c a b -> c (a b)")[:, 0: H2 * W2],
            in_=to[:],
        )
        stack.close()
        tc_cm.__exit__(None, None, None)
        return nc

    # ---------------- VQ ----------------
    # Reproduce the reference's fp32 expression tree exactly:
    #   d_j = (||z||^2 + ||cb_j||^2) - 2*(z . cb_j); argmin, first index on ties.
    # The big ||z||^2 add quantizes comparisons at ulp(~500) ~ 3e-5, which makes
    # the decision robust to our ~1e-6 z divergence -- but only if we use the
    # same formula. We compute nd = 2P - t1 (= -d bitwise) and argmax.
    zq = dram.tile([C, H2, W2], dec_dt, name="zq", tag="zq")
    NTOK = H2 * W2
    zbuf = smallpool.tile([C, NTOK], F32, name="zbuf", tag="zbuf")
    nc.sync.dma_start(out=zbuf[0:C, :], in_=z[:].rearrange("c a b -> c (a b)"))
    cbt_s = smallpool.tile([C, C], F32, name="cbt_s", tag="cbt_s")
    nc.sync.dma_start(out=cbt_s[:], in_=cbt_ext.ap())
    cb_f = smallpool.tile([C, C], F32, name="cb_f", tag="cb_f")
    nc.sync.dma_start(out=cb_f[:], in_=cb_ext.ap())
    cb_bf = smallpool.tile([C, C], dec_dt, name="cb_bf", tag="cb_bf")
    nc.vector.tensor_copy(out=cb_bf[:], in_=cb_f[:])
    ident = smallpool.tile([128, 128], F32, name="ident", tag="ident")
    make_identity(nc, ident[:])
    ones_t = smallpool.tile([C, 1], F32, name="ones_t", tag="ones_t")
    nc.vector.memset(ones_t[:], 1.0)

    TTOK = min(128, NTOK)
    cbnb = smallpool.tile([TTOK, C], F32, name="cbnb", tag="cbnb")
    nc.sync.dma_start(
        out=cbnb[:],
        in_=bass.AP(tensor=cbn_ext.ap().tensor, offset=0, ap=[[0, TTOK], [1, C]]),
    )
    rowsel = smallpool.tile([TTOK, C], F32, name="rowsel", tag="rowsel")
    nc.sync.dma_start(
        out=rowsel[:],
        in_=bass.AP(tensor=vqrow_ext.ap().tensor, offset=0, ap=[[0, TTOK], [1, C]]),
    )

    for tt in range(0, NTOK, TTOK):
        zchunk = zbuf[:, tt: tt + TTOK]
        p_ps = psum_small.tile([TTOK, C], F32, name="vq_p", tag="vq_ps", bufs=3)
        nc.tensor.matmul(p_ps[:], zchunk, cbt_s[:], start=True, stop=True)
        zsq = smallpool.tile([C, TTOK], F32, name="vq_zsq", tag="vq_zsq", bufs=2)
        nc.vector.tensor_tensor(out=zsq[:], in0=zchunk, in1=zchunk, op=ALU.mult)
        nrm_ps = psum_small.tile([TTOK, 1], F32, name="vq_nrm", tag="vq_ps",
                                 bufs=3)
        nc.tensor.matmul(nrm_ps[:], zsq[:], ones_t[:], start=True, stop=True)
        t1 = smallpool.tile([TTOK, C], F32, name="vq_t1", tag="vq_t1", bufs=2)
        nc.vector.tensor_scalar(
            out=t1[:], in0=cbnb[:], scalar1=nrm_ps[:], scalar2=None, op0=ALU.add
        )
        nd = smallpool.tile([TTOK, C], F32, name="vq_nd", tag="vq_nd", bufs=2)
        nc.vector.scalar_tensor_tensor(
            out=nd[:], in0=p_ps[:], scalar=2.0, in1=t1[:],
            op0=ALU.mult, op1=ALU.subtract,
        )
        mx = smallpool.tile([TTOK, 1], F32, name="vq_mx", tag="vq_mx", bufs=2)
        nc.vector.tensor_reduce(out=mx[:], in_=nd[:], axis=AX.X, op=ALU.max)
        oh = smallpool.tile([TTOK, C], F32, name="vq_oh", tag="vq_oh", bufs=2)
        nc.vector.tensor_scalar(
            out=oh[:], in0=nd[:], scalar1=mx[:], scalar2=None, op0=ALU.is_ge
        )
        tsel = smallpool.tile([TTOK, C], F32, name="vq_tsel", tag="vq_tsel",
                              bufs=2)
        nc.vector.tensor_tensor(out=tsel[:], in0=oh[:], in1=rowsel[:],
                                op=ALU.mult)
        mn = smallpool.tile([TTOK, 1], F32, name="vq_mn", tag="vq_mn", bufs=2)
        nc.vector.tensor_reduce(out=mn[:], in_=tsel[:], axis=AX.X, op=ALU.min)
        ohf = smallpool.tile([TTOK, C], F32, name="vq_ohf", tag="vq_ohf", bufs=2)
        nc.vector.tensor_scalar(
            out=ohf[:], in0=tsel[:], scalar1=mn[:], scalar2=None, op0=ALU.is_le
        )
        oht_ps = psum_small.tile([C, TTOK], F32, name="vq_oht", tag="vq_ps",
                                 bufs=3)
        nc.tensor.transpose(oht_ps[:], ohf[:], ident[0:TTOK, 0:TTOK])
        oht = smallpool.tile([C, TTOK], dec_dt, name="vq_ohtb", tag="vq_ohtb",
                             bufs=2)
        nc.vector.tensor_copy(out=oht[:], in_=oht_ps[:])
        zq_ps = psum_small.tile([C, TTOK], F32, name="vq_zq", tag="vq_ps", bufs=3)
        nc.tensor.matmul(zq_ps[:], cb_bf[:], oht[:], start=True, stop=True)
        zq_sb = smallpool.tile([C, TTOK], dec_dt, name="vq_zqsb", tag="vq_zqsb",
                               bufs=2)
        nc.vector.tensor_copy(out=zq_sb[:], in_=zq_ps[:])
        nc.sync.dma_start(
            out=zq[:].rearrange("c a b -> c (a b)")[:, tt: tt + TTOK],
            in_=zq_sb[:],
        )

    if stop_after == "vq":
        to2b = smallpool.tile([C, H2 * W2], dec_dt, name="dbg_vqb", tag="dbgb")
        nc.sync.dma_start(out=to2b[:], in_=zq[:].rearrange("c a b -> c (a b)"))
        to2 = smallpool.tile([C, H2 * W2], F32, name="dbg_vq", tag="dbg")
        nc.vector.tensor_copy(out=to2[:], in_=to2b[:])
        nc.sync.dma_start(
            out=out_ext.ap().rearrange("c a b -> c (a b)")[:, 0: H2 * W2],
            in_=to2[:],
        )
        stack.close()
        tc_cm.__exit__(None, None, None)
        return nc

    # ---------------- decoder ----------------
    up1 = dram.tile([C, H1, W1], dec_dt, name="up1", tag="up1")
    d0 = dram.tile([C, H1, W1], dec_dt, name="d0", tag="d0")
    rA = dram.tile([C, H1, W1], dec_dt, name="rA", tag="rA")
    d1 = dram.tile([C, H1, W1], dec_dt, name="d1", tag="d1")
    rB = dram.tile([C, H1, W1], dec_dt, name="rB", tag="rB")
    d2 = dram.tile([C, H1, W1], dec_dt, name="d2", tag="d2")
    up2 = dram.tile([C, H, W], dec_dt, name="up2", tag="up2")
    D0 = dram.tile([C, H, W], dec_dt, name="D0", tag="D0")
    RA = dram.tile([C, H, W], dec_dt, name="RA", tag="RA")
    D1 = dram.tile([C, H, W], dec_dt, name="D1", tag="D1")
    RB = dram.tile([C, H, W], dec_dt, name="RB", tag="RB")

    net.upsample_pass("up1", zq[:], up1[:], H2, W2, dec_dt)

    def dec_block(pref, up_src, conv_out, r_out, mid_out, r2_out, final_out,
                  HH, WW, final_dtype, final_stats=None):
        n_t = n_tiles(HH, WW)
        w_u = net.conv_weights(f"{pref}_conv", 3, dec_dt)
        st0 = stats_tile(f"{pref}_c", n_t)
        net.conv_pass(f"{pref}_conv", up_src, conv_out, w_u,
                      HH, WW, HH, WW, 1, dec_dt, stats=st0)
        g_a, bt_a = net.gn_weights(f"{pref}_r1gn1")
        s_a, t_a = net.gn_apply_params(f"{pref}_r1gn1", st0[0], st0[1], g_a,
                                       bt_a, pairmat2)
        w_1 = net.conv_weights(f"{pref}_r1c1", 3, dec_dt)
        st1 = stats_tile(f"{pref}_r1", n_t)
        net.conv_pass(f"{pref}_r1c1", conv_out, r_out, w_1,
                      HH, WW, HH, WW, 1, dec_dt, gn=(s_a, t_a), lrelu=True,
                      stats=st1)
        g_b, bt_b = net.gn_weights(f"{pref}_r1gn2")
        s_b, t_b = net.gn_apply_params(f"{pref}_r1gn2", st1[0], st1[1], g_b,
                                       bt_b, pairmat2)
        w_2 = net.conv_weights(f"{pref}_r1c2", 3, dec_dt)
        st2 = stats_tile(f"{pref}_m", n_t)
        net.conv_pass(f"{pref}_r1c2", r_out, mid_out, w_2,
                      HH, WW, HH, WW, 1, dec_dt, gn=(s_b, t_b), lrelu=True,
                      residual=conv_out, stats=st2)
        g_c, bt_c = net.gn_weights(f"{pref}_r2gn1")
        s_c, t_c = net.gn_apply_params(f"{pref}_r2gn1", st2[0], st2[1], g_c,
                                       bt_c, pairmat2)
        w_3 = net.conv_weights(f"{pref}_r2c1", 3, dec_dt)
        st3 = stats_tile(f"{pref}_r2", n_t)
        net.conv_pass(f"{pref}_r2c1", mid_out, r2_out, w_3,
                      HH, WW, HH, WW, 1, dec_dt, gn=(s_c, t_c), lrelu=True,
                      stats=st3)
        g_d, bt_d = net.gn_weights(f"{pref}_r2gn2")
        s_d, t_d = net.gn_apply_params(f"{pref}_r2gn2", st3[0], st3[1], g_d,
                                       bt_d, pairmat2)
        w_4 = net.conv_weights(f"{pref}_r2c2", 3, dec_dt)
        net.conv_pass(f"{pref}_r2c2", r2_out, final_out, w_4,
                      HH, WW, HH, WW, 1, dec_dt, gn=(s_d, t_d), lrelu=True,
                      residual=mid_out, stats=final_stats,
                      out_dtype=final_dtype)

    dec_block("dec1", up1[:], d0[:], rA[:], d1[:], rB[:], d2[:],
              H1, W1, dec_dt, final_stats=None)
    net.upsample_pass("up2", d2[:], up2[:], H1, W1, dec_dt)
    dec_block("dec2", up2[:], D0[:], RA[:], D1[:], RB[:], out_ext.ap(),
              H, W, F32, final_stats=None)

    stack.close()
    tc_cm.__exit__(None, None, None)
    return nc


# ---------------------------------------------------------------------------
# Host side
# ---------------------------------------------------------------------------

def _pack_conv_pair(w):
    """[co, ci, k, k] -> (wp [128, npair, co], ws [64, ns, co] or None)"""
    co, ci, kh, kw = w.shape
    k = kh
    wt = np.transpose(w, (1, 2, 3, 0)).astype(np.float32)  # [ci, ky, kx, co]
    pairs, singles = _taps_layout(k)
    wp = np.zeros((2 * ci, len(pairs), co), np.float32)
    for i, (ky, kx) in enumerate(pairs):
        wp[0:ci, i, :] = wt[:, ky, kx, :]
        wp[ci:, i, :] = wt[:, ky, kx + 1, :]
    ws = None
    if singles:
        ws = np.zeros((ci, len(singles), co), np.float32)
        for i, (ky, kx) in enumerate(singles):
            ws[:, i, :] = wt[:, ky, kx, :]
    return np.ascontiguousarray(wp), (
        np.ascontiguousarray(ws) if ws is not None else None
    )


def _flatten_params(params):
    out = {}

    def conv(name, cp):
        wp, ws = _pack_conv_pair(np.asarray(cp["w"]))
        out[f"{name}_wp"] = wp
        if ws is not None:
            out[f"{name}_ws"] = ws
        out[f"{name}_b"] = np.asarray(cp["b"], dtype=np.float32)

    def gn(name, gp):
        out[f"{name}_g"] = np.asarray(gp["g"], dtype=np.float32)
        out[f"{name}_bt"] = np.asarray(gp["b"], dtype=np.float32)

    conv("inconv", params["in_conv"])
    enc = params["enc"]
    conv("enc1_down", enc[0]["down"])
    gn("enc1_gn1", enc[0]["res"]["gn1"])
    conv("enc1_conv1", enc[0]["res"]["conv1"])
    gn("enc1_gn2", enc[0]["res"]["gn2"])
    conv("enc1_conv2", enc[0]["res"]["conv2"])
    conv("enc2_down", enc[1]["down"])
    gn("enc2_gn1", enc[1]["res"]["gn1"])
    conv("enc2_conv1", enc[1]["res"]["conv1"])
    gn("enc2_gn2", enc[1]["res"]["gn2"])
    conv("enc2_conv2", enc[1]["res"]["conv2"])

    cb = np.asarray(params["codebook"], dtype=np.float32)
    out["cb"] = cb
    out["cbt"] = np.ascontiguousarray(cb.T)
    out["cbn"] = (cb * cb).sum(1).astype(np.float32)
    out["vqrow"] = (np.arange(C) - 1e9).astype(np.float32)
    per = C // GROUPS
    pm = np.zeros((C2, C2), np.float32)
    for c2 in range(C2):
        c = c2 % C
        g0 = (c // per) * per
        for rep in range(2):
            pm[c2, rep * C + g0: rep * C + g0 + per] = 1.0 / (2 * per)
    # lhsT convention: out[m] = sum_k lhsT[k, m] * rhs[k] -> build transpose
    out["pairmat2"] = np.ascontiguousarray(pm.T)

    dec = params["dec"]
    for i, pref in enumerate(["dec1", "dec2"]):
        blk = dec[i]
        conv(f"{pref}_conv", blk["conv"])
        gn(f"{pref}_r1gn1", blk["res1"]["gn1"])
        conv(f"{pref}_r1c1", blk["res1"]["conv1"])
        gn(f"{pref}_r1gn2", blk["res1"]["gn2"])
        conv(f"{pref}_r1c2", blk["res1"]["conv2"])
        gn(f"{pref}_r2gn1", blk["res2"]["gn1"])
        conv(f"{pref}_r2c1", blk["res2"]["conv1"])
        gn(f"{pref}_r2gn2", blk["res2"]["gn2"])
        conv(f"{pref}_r2c2", blk["res2"]["conv2"])
    return out


def _make_in_maps(x, params):
    p = _flatten_params(params)
    maps = []
    for i in range(N_CORES):
        m = dict(p)
        m["x"] = np.ascontiguousarray(np.asarray(x)[i], dtype=np.float32)
        maps.append(m)
    return maps


_CACHED = {}


def _ensure_ntff_hook():
    try:
        from antenv.axon_hooks import get_axon_ntff_profile_hook  # noqa: F401
        return
    except ImportError:
        pass
    try:
        import types
        import antenv
        from trn_agent_boot.trn_boot import _ntff_profile_via_ctypes

        hook = _ntff_profile_via_ctypes("/opt/axon/libaxon_pjrt.so")
        mod = types.ModuleType("antenv.axon_hooks")
        _h = [hook]
        mod.set_axon_ntff_profile_hook = lambda h: _h.__setitem__(0, h)
        mod.get_axon_ntff_profile_hook = lambda: _h[0]
        sys.modules["antenv.axon_hooks"] = mod
        antenv.axon_hooks = mod
    except Exception as e:  # pragma: no cover
        print("ntff hook install failed:", e)


def _run(x, params, trace=False):
    if trace:
        _ensure_ntff_hook()
    x = np.asarray(x)
    key = "net"
    if key not in _CACHED:
        _CACHED[key] = build_net()
    nc = _CACHED[key]
    in_maps = _make_in_maps(x, params)
    res = run_bass_kernel_spmd(
        nc, in_maps, core_ids=list(range(N_CORES)), trace=trace
    )
    out = np.stack([res.results[i]["out"] for i in range(N_CORES)], axis=0)
    return out, res.exec_time_ns


def kernel(x, params):
    return _run(x, params, trace=False)[0]


def kernel_timed(x, params):
    return _run(x, params, trace=True)
